# revision 1
# baseline (speedup 1.0000x reference)
"""Trainium2 Bass kernel for nn_BarrierPolicy (CBF-QP safety filter).

Data-parallel over batch: 8 cores x 32768 samples.
Phase A (per 2048-sample tile): load x in "xview" layout, PE-transpose to
"SP2" (stacked pack-2) layout, run the 3-layer MLP + dynamics matmuls on the
tensor engine in bf16 (1 cycle/row), transpose results back to xview.
Bias-add for px and the sigmoid for alpha are deferred to xview where they
are single wide ops instead of many narrow ones.
Phase B (per 1024-col chunk, 2 chunks): Kiwiel variable-fixing active-set
solve of the per-sample box-QP dual (3 iterations + closed-form finish) in
bf16 elementwise / f32 slot math, then u = clip(-p + lam*g) in f32.
Broadcast of per-sample scalars over the 8 coords is materialized by the
scalar (ACT) engine; slot math mostly on GPSIMD to keep DVE for the wide
bf16 elementwise ops.

Layouts (per tile of 2048 samples):
  xview: SBUF (128, 128): partition r, col 16b+8s0+j <-> sample 256b+2r+s0, coord j
  SP2  : transpose of xview: partition 16b+8s0+j, col r
  padded psum (for 16-row matmul outs, 32-align rule): block b at partitions
  [32(b%3), 32(b%3)+16), free-slot b//3.
  slot : per-sample scalars (128, 256): partition r, col 2b+s0 per tile
"""
import numpy as np

B_FULL, N = 262144, 8
NCORES = 8
S = B_FULL // NCORES          # 32768 samples per core
TILE = 2048
NT = S // TILE                # 16 tiles
NSLOT = S // 128              # 256 slot cols per core
NCH = 4                       # phase-B chunks
TPC = NT // NCH               # tiles per chunk
FC = S // 16                  # 2048 xview cols per core
T_KIWIEL = 3
LAMCAP = float(2.0 ** 40)
EPS = 1e-12

_CACHE = {}

_CSHAPES_BF = dict(TL2=(128, 128), TL3px=(64, 16), TL3a=(128, 2),
                   TDA=(128, 128), TDG=(128, 128), ID128H=(128, 128),
                   B31J=(128, 16),
                   **{f"TL1E{b}": (128, 128) for b in range(8)})
_CSHAPES_F32 = dict(ID128=(128, 128), B1v=(128, 1), B2v=(128, 1),
                    B32s=(128, 1))


def _consts(W1, b1, W21, b21, W22, b22, W31, b31, W32, b32, A, G):
    import ml_dtypes
    f32 = np.float32
    bf = ml_dtypes.bfloat16
    out = {}
    for b in range(8):
        T = np.zeros((128, 128), f32)
        for s0 in range(2):
            T[16 * b + 8 * s0:16 * b + 8 * s0 + 8, 64 * s0:64 * s0 + 64] = W1
        out[f"TL1E{b}"] = T.astype(bf)
    TL2 = np.zeros((128, 128), f32)
    for s0 in range(2):
        TL2[64 * s0:64 * s0 + 64, 32 * s0:32 * s0 + 32] = W21
        TL2[64 * s0:64 * s0 + 64, 64 + 32 * s0:64 + 32 * s0 + 32] = W22
    TL3px = np.zeros((64, 16), f32)
    for s0 in range(2):
        TL3px[32 * s0:32 * s0 + 32, 8 * s0:8 * s0 + 8] = W31
    TL3a = np.zeros((128, 2), f32)          # used as slice [64:128)
    for s0 in range(2):
        TL3a[64 + 32 * s0:64 + 32 * s0 + 32, s0:s0 + 1] = W32
    TDA = np.kron(np.eye(16, dtype=f32), A.T.astype(f32))         # out = A x
    TDG = np.kron(np.eye(16, dtype=f32), (-2.0 * G).astype(f32))  # out = -2 G^T x
    # per-coordinate b31 bias: col 8s+j -> b31[j]
    B31J = np.tile(b31.astype(f32), 2)[None, :].repeat(128, 0)
    out.update(TL2=TL2.astype(bf), TL3px=TL3px.astype(bf), TL3a=TL3a.astype(bf),
               TDA=TDA.astype(bf), TDG=TDG.astype(bf),
               ID128H=np.eye(128, dtype=f32).astype(bf),
               B31J=B31J.astype(bf))
    out["ID128"] = np.eye(128, dtype=f32)
    out["B1v"] = np.concatenate([b1, b1]).reshape(128, 1).astype(f32)
    out["B2v"] = np.concatenate([b21, b21, b22, b22]).reshape(128, 1).astype(f32)
    out["B32s"] = np.full((128, 1), float(b32[0]), f32)
    return out


def build_kernel(nc, tc, x_d, u_d, cds):
    from concourse import mybir
    f32 = mybir.dt.float32
    f32r = mybir.dt.float32r
    bf16 = mybir.dt.bfloat16
    AL = mybir.AluOpType
    AF = mybir.ActivationFunctionType
    XL = mybir.AxisListType.X
    V, GP, SC = nc.vector, nc.gpsimd, nc.scalar

    with (
        tc.tile_pool(name="const", bufs=1) as cpool,
        tc.tile_pool(name="pers", bufs=1) as pers,
        tc.tile_pool(name="work", bufs=2) as work,
        tc.tile_pool(name="psA", bufs=1, space="PSUM") as psA,
        tc.tile_pool(name="psB", bufs=1, space="PSUM") as psB,
    ):
        C = {}
        for k, v in _CSHAPES_BF.items():
            C[k] = cpool.tile(list(v), bf16, tag=k, name=k)
        for k, v in _CSHAPES_F32.items():
            C[k] = cpool.tile(list(v), f32, tag=k, name=k)
        for k in list(_CSHAPES_BF) + list(_CSHAPES_F32):
            nc.sync.dma_start(C[k][:], cds[k][:])

        def fc_f32(tag):
            return pers.tile([128, FC], f32, tag=tag, name=tag)

        def fc_bf(tag):
            return pers.tile([128, FC], bf16, tag=tag, name=tag)

        def sl_tile(tag):
            return pers.tile([128, NSLOT], f32, tag=tag, name=tag)

        x_xv, u32 = fc_f32("x_xv"), fc_f32("u32")
        p_xv, g_xv = fc_bf("p_xv"), fc_bf("g_xv")
        gt_xv, pt_xv, q_xv = fc_bf("gt_xv"), fc_bf("pt_xv"), fc_bf("q_xv")
        zt_xv, mm_xv = fc_bf("zt_xv"), fc_bf("mm_xv")
        sc1, sc2, sc3 = fc_bf("sc1"), fc_bf("sc2"), fc_bf("sc3")
        lbc, bvbc = fc_bf("lbc"), fc_bf("bvbc")
        araw = sl_tile("araw")
        alpha4, lfhx, sxx = sl_tile("alpha4"), sl_tile("lfhx"), sl_tile("sxx")
        c0s, viol, nviol, infs = (sl_tile("c0s"), sl_tile("viol"),
                                  sl_tile("nviol"), sl_tile("infs"))
        nums, dens, lams = sl_tile("nums"), sl_tile("dens"), sl_tile("lams")
        t1s, t2s = sl_tile("t1s"), sl_tile("t2s")

        # ---------------- Phase A ----------------
        # px/alpha matmul outs leave pad regions unwritten; zero once so the
        # full-tile evac copies and transposes never see uninitialized PSUM.
        LPx = psA.tile([128, 2, 128], f32, tag="LPx", name="LPx")
        alP = psA.tile([128, 2, 128], f32, tag="alP", name="alP")
        V.memset(LPx[:], 0.0)
        V.memset(alP[:], 0.0)
        for t in range(NT):
            cs = slice(128 * t, 128 * t + 128)
            ss = slice(16 * t, 16 * t + 16)
            nc.sync.dma_start(
                x_xv[:, cs].rearrange("p (b s j) -> p b s j", b=8, s=2, j=8),
                x_d[t * TILE:(t + 1) * TILE, :].rearrange(
                    "(b r s) j -> r b s j", b=8, r=128, s=2))
            TP = psA.tile([128, 3, 128], f32, tag="TP", name="TP")
            nc.tensor.transpose(TP[:, 0, :], x_xv[:, cs], C["ID128"][:])
            xsp2 = work.tile([128, 128], bf16, tag="xsp2", name="xsp2")
            V.tensor_copy(xsp2[:], TP[:, 0, :])

            h1P = psA.tile([128, 4, 128], f32, tag="h1P", name="h1P")
            x2P = psA.tile([128, 4, 128], f32, tag="x2P", name="x2P")
            h1 = work.tile([128, 8, 128], bf16, tag="h1", name="h1")
            x2 = work.tile([128, 8, 128], bf16, tag="x2", name="x2")

            for half in range(2):
                hs = slice(4 * half, 4 * half + 4)
                for bi in range(4):
                    b = 4 * half + bi
                    nc.tensor.matmul(h1P[:, bi, :], C[f"TL1E{b}"][:], xsp2[:])
                # relu + bias evac, one wide op per half (PSUM: DVE/ACT only)
                SC.activation(h1[:, hs, :], h1P[:], AF.Relu, bias=C["B1v"][:])
                for bi in range(4):
                    b = 4 * half + bi
                    nc.tensor.matmul(x2P[:, bi, :], C["TL2"][:], h1[:, b, :])
                SC.activation(x2[:, hs, :], x2P[:], AF.Relu, bias=C["B2v"][:])
                for bi in range(4):
                    b = 4 * half + bi
                    m4, k2 = b % 4, b // 4
                    nc.tensor.matmul(LPx[32 * m4:32 * m4 + 16, k2, :],
                                     C["TL3px"][:], x2[0:64, b, :],
                                     tile_position=(0, 32 * m4))
                    nc.tensor.matmul(alP[32 * m4:32 * m4 + 2, k2, :],
                                     C["TL3a"][64:128, :], x2[64:128, b, :],
                                     tile_position=(64, 32 * m4))

            # raw px / alpha evac (bias+sigmoid deferred to xview)
            pxe = work.tile([128, 2, 128], bf16, tag="pxe", name="pxe")
            asle = work.tile([128, 2, 128], bf16, tag="asle", name="asle")
            SC.activation(pxe[:], LPx[:], AF.Copy)
            SC.activation(asle[:], alP[:], AF.Copy)

            nc.tensor.matmul(TP[:, 1, :], C["TDA"][:], xsp2[:])
            nc.tensor.matmul(TP[:, 2, :], C["TDG"][:], xsp2[:])
            axs = work.tile([128, 128], bf16, tag="axs", name="axs")
            gsp2 = work.tile([128, 128], bf16, tag="gsp2", name="gsp2")
            V.tensor_copy(axs[:], TP[:, 1, :])
            V.tensor_copy(gsp2[:], TP[:, 2, :])

            # transposes back to xview
            trP = psB.tile([128, 2, 128], bf16, tag="trP", name="trP")
            pxtP = psB.tile([128, 2, 128], bf16, tag="pxtP", name="pxtP")
            altP = psB.tile([128, 2, 128], bf16, tag="altP", name="altP")
            nc.tensor.transpose(trP[:, 0, :], gsp2[:], C["ID128H"][:])
            nc.tensor.transpose(trP[:, 1, :], axs[:], C["ID128H"][:])
            V.tensor_copy(g_xv[:, cs], trP[:, 0, :])
            prodA = work.tile([128, 128], f32, tag="prodA", name="prodA")
            V.scalar_tensor_tensor(prodA[:], trP[:, 1, :], -2.0,
                                   x_xv[:, cs], AL.mult, AL.mult)
            V.tensor_reduce(lfhx[:, ss],
                            prodA[:].rearrange("p (c j) -> p c j", j=8),
                            XL, AL.add)
            sqx = work.tile([128, 128], f32, tag="sqx", name="sqx")
            GP.tensor_tensor(sqx[:], x_xv[:, cs], x_xv[:, cs], AL.mult)
            V.tensor_reduce(sxx[:, ss],
                            sqx[:].rearrange("p (c j) -> p c j", j=8),
                            XL, AL.add)

            for k in range(2):
                nc.tensor.transpose(pxtP[:, k, :], pxe[:, k, :], C["ID128H"][:])
                nc.tensor.transpose(altP[:, k, :], asle[:, k, :], C["ID128H"][:])
            # px (+ b31 bias) and alpha back to xview, one wide op each
            dstp = p_xv[:, cs].rearrange("p (k m sj) -> p k m sj",
                                         k=2, m=4, sj=16)
            srcp = pxtP.rearrange("p k (m g sj) -> p k m g sj",
                                     m=4, g=2, sj=16)[:, :, :, 0, :]
            V.tensor_tensor(
                dstp, srcp,
                C["B31J"][:].rearrange("p (k m sj) -> p k m sj",
                                       k=1, m=1, sj=16)
                .broadcast_to((128, 2, 4, 16)), AL.add)
            dsta = araw[:, ss].rearrange("p (k m s) -> p k m s", k=2, m=4, s=2)
            srca = altP.rearrange("p k (m g) -> p k m g",
                                     m=4, g=32)[:, :, :, 0:2]
            V.tensor_copy(dsta, srca)

        # ---------------- Phase B (per chunk) ----------------
        CF = FC // NCH          # 1024 fc cols per chunk
        CL = NSLOT // NCH       # 128 slot cols per chunk
        x3 = lambda ap: ap.rearrange("p (c j) -> p c j", j=8)

        for ch in range(NCH):
            fs = slice(CF * ch, CF * ch + CF)
            sl = slice(CL * ch, CL * ch + CL)
            pF, gF = p_xv[:, fs], g_xv[:, fs]
            gtF, ptF, qF = gt_xv[:, fs], pt_xv[:, fs], q_xv[:, fs]
            ztF, mmF = zt_xv[:, fs], mm_xv[:, fs]
            s1F, s2F, s3F = sc1[:, fs], sc2[:, fs], sc3[:, fs]
            lbcF, bvbcF = lbc[:, fs], bvbc[:, fs]
            u32F = u32[:, fs]
            c0L, viL, nviL, inL = c0s[:, sl], viol[:, sl], nviol[:, sl], infs[:, sl]
            nmL, dnL, lmL = nums[:, sl], dens[:, sl], lams[:, sl]
            t1L, t2L = t1s[:, sl], t2s[:, sl]
            arL, a4L = araw[:, sl], alpha4[:, sl]
            bcv = lambda apL: apL.broadcast_to((128, CL, 8))

            # alpha sigmoid (deferred from phase A; b31 bias folded into evac)
            SC.activation(a4L, arL, AF.Sigmoid, bias=C["B32s"][:])

            # c0 = Lfhx + 4*sigm*(16 - sxx);  (alpha4 holds the sigmoid)
            GP.tensor_scalar(t1L, sxx[:, sl], -1.0, 16.0, AL.mult, AL.add)
            GP.tensor_tensor(t2L, a4L, t1L, AL.mult)
            GP.tensor_scalar(t2L, t2L, 4.0, None, AL.mult)
            GP.tensor_tensor(c0L, t2L, lfhx[:, sl], AL.add)

            # transformed QP data
            SC.sign(s1F, gF)                                   # sigma
            V.tensor_tensor(ptF, s1F, pF, AL.mult)             # pt = sigma*p
            V.tensor_scalar(ztF, ptF, -1.0, None, AL.mult)     # zt0 = -pt
            SC.activation(gtF, gF, AF.Abs)
            SC.activation(qF, gF, AF.Square)
            GP.memset(mmF, 1.0)

            # c(0) and feasibility
            V.tensor_scalar(s2F, ztF, 1.0, -1.0, AL.min, AL.max)   # u0
            V.tensor_tensor(s1F, gtF, s2F, AL.mult)
            V.tensor_reduce(t1L, x3(s1F), XL, AL.add)
            GP.tensor_tensor(t1L, c0L, t1L, AL.add)
            GP.tensor_scalar(viL, t1L, 0.0, None, AL.is_lt)
            GP.tensor_scalar(nviL, viL, -1.0, None, AL.mult)
            V.tensor_reduce(t2L, x3(gtF), XL, AL.add)
            GP.tensor_tensor(t2L, c0L, t2L, AL.add)
            GP.tensor_scalar(inL, t2L, 0.0, None, AL.is_lt)
            GP.tensor_tensor(inL, inL, viL, AL.mult)

            # initial num/den (zt = -pt, mm = 1)
            V.tensor_tensor(s1F, gtF, ztF, AL.mult)
            V.tensor_reduce(nmL, x3(s1F), XL, AL.add)
            GP.tensor_tensor(nmL, c0L, nmL, AL.add)
            V.tensor_reduce(dnL, x3(qF), XL, AL.add)

            def calc_lam():
                GP.tensor_scalar(t1L, dnL, EPS, None, AL.add)
                V.reciprocal(t2L, t1L)
                GP.tensor_tensor(lmL, nmL, t2L, AL.mult)
                GP.tensor_tensor(lmL, lmL, nviL, AL.mult)      # lam = -num/den*viol

            calc_lam()
            for _ in range(T_KIWIEL):
                SC.activation(x3(lbcF), bcv(lmL), AF.Copy)         # lam bcast
                V.tensor_tensor(s2F, lbcF, gtF, AL.mult)
                V.tensor_tensor(s2F, s2F, ptF, AL.subtract)        # ur
                V.tensor_scalar(s2F, s2F, 1.0, -1.0, AL.min, AL.max)  # uhat
                V.tensor_tensor(s1F, gtF, s2F, AL.mult)
                V.tensor_reduce(t1L, x3(s1F), XL, AL.add)
                GP.tensor_tensor(t1L, c0L, t1L, AL.add)            # c
                GP.tensor_scalar(t2L, t1L, -1.0, None, AL.mult)    # -c
                SC.activation(x3(bvbcF), bcv(t2L), AF.Sign)        # bvs = sign(-c)
                V.tensor_tensor(s1F, bvbcF, s2F, AL.mult)
                V.tensor_scalar(s1F, s1F, 1.0, None, AL.is_ge)
                V.tensor_tensor(s1F, s1F, mmF, AL.mult)            # fix
                GP.tensor_tensor(s3F, bvbcF, ztF, AL.subtract)
                V.tensor_tensor(s3F, s1F, s3F, AL.mult)
                V.tensor_tensor(ztF, ztF, s3F, AL.add)
                GP.tensor_tensor(mmF, mmF, s1F, AL.subtract)
                V.tensor_tensor(s1F, gtF, ztF, AL.mult)
                V.tensor_reduce(nmL, x3(s1F), XL, AL.add)
                GP.tensor_tensor(nmL, c0L, nmL, AL.add)
                V.tensor_tensor(s1F, qF, mmF, AL.mult)
                V.tensor_reduce(dnL, x3(s1F), XL, AL.add)
                calc_lam()

            # infeasible rows -> lam = LAMCAP
            GP.tensor_scalar(t1L, lmL, -1.0, LAMCAP, AL.mult, AL.add)
            GP.tensor_tensor(t1L, t1L, inL, AL.mult)
            GP.tensor_tensor(lmL, lmL, t1L, AL.add)
            SC.activation(x3(lbcF), bcv(lmL), AF.Copy)
            V.tensor_tensor(s1F, lbcF, gF, AL.mult)
            V.tensor_tensor(s1F, s1F, pF, AL.subtract)
            V.tensor_scalar(u32F, s1F, 1.0, -1.0, AL.min, AL.max)
            for tt in range(TPC * ch, TPC * (ch + 1)):
                nc.sync.dma_start(
                    u_d[tt * TILE:(tt + 1) * TILE, :].rearrange(
                        "(b r s) j -> r b s j", b=8, r=128, s=2),
                    u32[:, 128 * tt:128 * tt + 128].rearrange(
                        "p (b s j) -> p b s j", b=8, s=2, j=8))


def _build():
    from concourse import bacc, mybir
    from concourse import tile as tile_mod
    from concourse._compat import axon_active
    f32 = mybir.dt.float32
    bf16 = mybir.dt.bfloat16
    nc = bacc.Bacc("TRN2", target_bir_lowering=False,
                   debug=not axon_active(), num_devices=NCORES)
    x_d = nc.dram_tensor("x", [S, N], f32, kind="ExternalInput").ap()
    u_d = nc.dram_tensor("u", [S, N], f32, kind="ExternalOutput").ap()
    cds = {}
    for k, v in _CSHAPES_BF.items():
        cds[k] = nc.dram_tensor(k, list(v), bf16, kind="ExternalInput").ap()
    for k, v in _CSHAPES_F32.items():
        cds[k] = nc.dram_tensor(k, list(v), f32, kind="ExternalInput").ap()
    with tile_mod.TileContext(nc) as tc:
        build_kernel(nc, tc, x_d, u_d, cds)
    nc.compile()
    return nc


def kernel(x, W1, b1, W21, b21, W22, b22, W31, b31, W32, b32, A, G, mean, std):
    from concourse.bass_utils import run_bass_kernel_spmd
    f32 = np.float32
    x = np.asarray(x, f32)
    x0 = (x * np.asarray(std, f32) + np.asarray(mean, f32)).astype(f32)

    consts = _consts(np.asarray(W1, f32), np.asarray(b1, f32), np.asarray(W21, f32),
                     np.asarray(b21, f32), np.asarray(W22, f32), np.asarray(b22, f32),
                     np.asarray(W31, f32), np.asarray(b31, f32), np.asarray(W32, f32),
                     np.asarray(b32, f32), np.asarray(A, f32), np.asarray(G, f32))
    if "nc" not in _CACHE:
        _CACHE["nc"] = _build()
    nc = _CACHE["nc"]

    in_maps = []
    for c in range(NCORES):
        m = {"x": np.ascontiguousarray(x0[c * S:(c + 1) * S])}
        m.update(consts)
        in_maps.append(m)
    res = run_bass_kernel_spmd(nc, in_maps, list(range(NCORES)))
    out = np.concatenate([np.asarray(res.results[c]["u"]) for c in range(NCORES)],
                         axis=0)
    return out.astype(f32)



# revision 5
# speedup vs baseline: 1.3971x; 1.3971x over previous
"""Trainium2 Bass kernel for nn_BarrierPolicy (CBF-QP safety filter), v2.

Data-parallel over batch: 8 cores x 32768 samples, all math bf16 on-chip.

Phase A (per 4-tile super-block of 8192 samples): x arrives bf16 in xview
layout (partition r, col 16b+8s0+j), PE-transposes to SP2, runs the MLP +
dynamics matmuls with 512-wide moving dim (few, fat matmuls; bf16 PSUM
outputs), evacuates with relu+bias as wide ops split across ACT/Pool/DVE,
transposes px/g/(-2Ax) back to xview in batched PSUM banks.

Phase B (per 8-tile chunk, pipelined behind phase A): Newton-form Kiwiel
variable-fixing for the box-QP dual:
  lam' = clip(lam - c(lam)/den, 0, LAMCAP),  den = sum of q over the
  not-yet-fixed set; coords are permanently fixed one-sided (uhat == bvs,
  bvs = sign(-c)). Clip via 4x tensor_scalar, per-sample reductions via
  2-level bf16 tree-add + f32 final, per-sample scalars broadcast through a
  duplicated-pair view that keeps DVE in 2x mode. T_NEWTON iterations with
  closed-form final u = clip(lam*g - p).
"""
import numpy as np

B_FULL, N = 262144, 8
NCORES = 8
S = B_FULL // NCORES          # 32768 samples per core
TILE = 2048
NT = S // TILE                # 16 tiles
SUP = 4                       # tiles per super-block
NSUP = NT // SUP
MV = 128 * SUP                # matmul moving width per super (512)
FC = S // 16                  # 2048 xview cols per core
NSLOT = S // 128              # 256 slot cols per core
NCH = 2                       # phase-B chunks
CF = FC // NCH                # 1024
CL = NSLOT // NCH             # 128
SPC = NSUP // NCH             # supers per chunk
T_NEWTON = 3
LAMCAP = float(2.0 ** 40)
EPS = 1e-9

_CACHE = {}

_CSHAPES_BF = dict(TL2=(128, 128), TL3S=(128, 32), TDA=(128, 128),
                   TDG=(128, 128), ID128H=(128, 128),
                   **{f"TL1E{b}": (128, 128) for b in range(8)})
_CSHAPES_F32 = dict(B1v=(128, 1), B2v=(128, 1), B3v=(128, 1))


def _consts(W1, b1, W21, b21, W22, b22, W31, b31, W32, b32, A, G):
    import ml_dtypes
    f32 = np.float32
    bf = ml_dtypes.bfloat16
    out = {}
    for b in range(8):
        T = np.zeros((128, 128), f32)
        for s0 in range(2):
            T[16 * b + 8 * s0:16 * b + 8 * s0 + 8, 64 * s0:64 * s0 + 64] = W1
        out[f"TL1E{b}"] = T.astype(bf)
    TL2 = np.zeros((128, 128), f32)
    for s0 in range(2):
        TL2[64 * s0:64 * s0 + 64, 32 * s0:32 * s0 + 32] = W21
        TL2[64 * s0:64 * s0 + 64, 64 + 32 * s0:64 + 32 * s0 + 32] = W22
    # stacked L3: out col m = 16*s0 + mm, mm in 0..7 -> px_j, mm=8 -> alpha raw
    TL3S = np.zeros((128, 32), f32)
    for s0 in range(2):
        TL3S[32 * s0:32 * s0 + 32, 16 * s0:16 * s0 + 8] = W31
        TL3S[64 + 32 * s0:64 + 32 * s0 + 32, 16 * s0 + 8:16 * s0 + 9] = W32
    TDA = np.kron(np.eye(16, dtype=f32), (-2.0 * A.T).astype(f32))  # -2 A x
    TDG = np.kron(np.eye(16, dtype=f32), (-2.0 * G).astype(f32))    # -2 G^T x
    out.update(TL2=TL2.astype(bf), TL3S=TL3S.astype(bf),
               TDA=TDA.astype(bf), TDG=TDG.astype(bf),
               ID128H=np.eye(128, dtype=f32).astype(bf))
    out["B1v"] = np.concatenate([b1, b1]).reshape(128, 1).astype(f32)
    out["B2v"] = np.concatenate([b21, b21, b22, b22]).reshape(128, 1).astype(f32)
    b3 = np.concatenate([b31.astype(f32), [np.float32(b32[0])], np.zeros(7, f32)])
    out["B3v"] = np.tile(b3, 8).reshape(128, 1).astype(f32)
    return out


def build_kernel(nc, tc, x_d, u_d, cds):
    from concourse import mybir
    f32 = mybir.dt.float32
    bf16 = mybir.dt.bfloat16
    AL = mybir.AluOpType
    AF = mybir.ActivationFunctionType
    V, GP, SC = nc.vector, nc.gpsimd, nc.scalar

    with (
        tc.tile_pool(name="const", bufs=1) as cpool,
        tc.tile_pool(name="pers", bufs=1) as pers,
        tc.tile_pool(name="work", bufs=2) as work,
        tc.tile_pool(name="psT", bufs=2, space="PSUM") as psT,
        tc.tile_pool(name="psM", bufs=2, space="PSUM") as psM,
        tc.tile_pool(name="psX", bufs=1, space="PSUM") as psX,
    ):
        C = {}
        for k, v in _CSHAPES_BF.items():
            C[k] = cpool.tile(list(v), bf16, tag=k, name=k)
        for k, v in _CSHAPES_F32.items():
            C[k] = cpool.tile(list(v), f32, tag=k, name=k)
        for k in list(_CSHAPES_BF) + list(_CSHAPES_F32):
            nc.sync.dma_start(C[k][:], cds[k][:])

        def fc_bf(tag):
            return pers.tile([128, FC], bf16, tag=tag, name=tag)

        def sl_f32(tag):
            return pers.tile([128, NSLOT], f32, tag=tag, name=tag)

        # persistent full-width tensors (xview layout)
        xvb = fc_bf("xvb")
        g_xv, px_xv = fc_bf("g_xv"), fc_bf("px_xv")
        gt, pt, qq = fc_bf("gt"), fc_bf("pt"), fc_bf("qq")
        sgn = fc_bf("sgn")
        ur, uh, rb = fc_bf("ur"), fc_bf("uh"), fc_bf("rb")
        nf = fc_bf("nf")
        prodA, sqx = fc_bf("prodA"), fc_bf("sqx")
        uout = fc_bf("uout")
        l1a = pers.tile([128, FC // 2], bf16, tag="l1a", name="l1a")
        l2a = pers.tile([128, FC // 4], bf16, tag="l2a", name="l2a")
        l1b = pers.tile([128, FC // 2], bf16, tag="l1b", name="l1b")
        l2b = pers.tile([128, FC // 4], bf16, tag="l2b", name="l2b")
        # per-sample slots (f32) and dup-pair broadcasts (bf16)
        c0s, lam, lfh, sxx = sl_f32("c0s"), sl_f32("lam"), sl_f32("lfh"), sl_f32("sxx")
        csum, dsum, cc = sl_f32("csum"), sl_f32("dsum"), sl_f32("cc")
        rr, st = sl_f32("rr"), sl_f32("st")
        araw = pers.tile([128, NSLOT], bf16, tag="araw", name="araw")
        al4 = pers.tile([128, NSLOT], bf16, tag="al4", name="al4")
        lam2 = pers.tile([128, NSLOT, 2], bf16, tag="lam2", name="lam2")
        bvs2 = pers.tile([128, NSLOT, 2], bf16, tag="bvs2", name="bvs2")

        x8 = lambda ap: ap.rearrange("p (c j) -> p c j", j=8)
        x4 = lambda ap: ap.rearrange("p (c j) -> p c j", j=4)
        x2v = lambda ap: ap.rearrange("p (c j) -> p c j", j=2)

        def tree8(src, out_f32, sl, eng_last=GP):
            """out[:, sl] = sum over j=8 of src[:, 8*sl]: 2 bf16 levels + f32."""
            fs = slice(sl.start * 8, sl.stop * 8)
            h1s = slice(sl.start * 4, sl.stop * 4)
            h2s = slice(sl.start * 2, sl.stop * 2)
            la, lb = (l1a, l2a) if src is not qq else (l1b, l2b)
            s8 = x8(src[:, fs])
            V.tensor_tensor(x4(la[:, h1s]), s8[:, :, 0:4], s8[:, :, 4:8], AL.add)
            V.tensor_tensor(x2v(lb[:, h2s]), x4(la[:, h1s])[:, :, 0:2],
                            x4(la[:, h1s])[:, :, 2:4], AL.add)
            eng_last.tensor_tensor(out_f32[:, sl], x2v(lb[:, h2s])[:, :, 0],
                                   x2v(lb[:, h2s])[:, :, 1], AL.add)

        # ---------------- Phase A (per super-block) ----------------
        def phase_a(sp):
            cs = slice(MV * sp, MV * sp + MV)               # xview cols
            ss = slice(16 * SUP * sp, 16 * SUP * (sp + 1))  # slot cols
            TPx = psT.tile([128, SUP, 128], bf16, tag="TPx", name="TPx")
            for t in range(SUP):
                tt = SUP * sp + t
                nc.sync.dma_start(
                    xvb[:, 128 * tt:128 * tt + 128].rearrange(
                        "p (b s j) -> p b s j", b=8, s=2, j=8),
                    x_d[tt * TILE:(tt + 1) * TILE, :].rearrange(
                        "(b r s) j -> r b s j", b=8, r=128, s=2))
                nc.tensor.transpose(TPx[:, t, :], xvb[:, 128 * tt:128 * tt + 128],
                                    C["ID128H"][:])
            xsp2 = work.tile([128, MV], bf16, tag="xsp2", name="xsp2")
            V.tensor_copy(xsp2[:].rearrange("p (t c) -> p t c", t=SUP), TPx[:])

            h1 = work.tile([128, 8, MV], bf16, tag="h1", name="h1")
            x2 = work.tile([128, 8, MV], bf16, tag="x2", name="x2")
            ev1 = [SC, GP, SC, V]
            ev2 = [SC, GP, SC, V]
            for qr in range(4):
                mmP = psM.tile([128, 2, MV], f32, tag="mmP", name="mmP")
                for bi in range(2):
                    nc.tensor.matmul(mmP[:, bi, :], C[f"TL1E{2 * qr + bi}"][:],
                                     xsp2[:])
                hs = slice(2 * qr, 2 * qr + 2)
                e = ev1[qr]
                if e is SC:
                    SC.activation(h1[:, hs, :], mmP[:], AF.Relu, bias=C["B1v"][:])
                else:
                    e.tensor_scalar(h1[:, hs, :], mmP[:], C["B1v"][:], 0.0,
                                    AL.add, AL.max)
            for qr in range(4):
                mmP = psM.tile([128, 2, MV], f32, tag="mmP", name="mmP")
                for bi in range(2):
                    nc.tensor.matmul(mmP[:, bi, :], C["TL2"][:],
                                     h1[:, 2 * qr + bi, :])
                hs = slice(2 * qr, 2 * qr + 2)
                e = ev2[qr]
                if e is SC:
                    SC.activation(x2[:, hs, :], mmP[:], AF.Relu, bias=C["B2v"][:])
                else:
                    e.tensor_scalar(x2[:, hs, :], mmP[:], C["B2v"][:], 0.0,
                                    AL.add, AL.max)

            LA = psM.tile([128, 2, MV], f32, tag="mmP", name="LA")
            for b in range(8):
                g4, k2 = b % 4, b // 4
                nc.tensor.matmul(LA[32 * g4:32 * g4 + 32, k2, :], C["TL3S"][:],
                                 x2[:, b, :], tile_position=(0, 32 * g4))
            pxal = work.tile([128, 2, MV], bf16, tag="pxal", name="pxal")
            SC.activation(pxal[:], LA[:], AF.Lrelu, bias=C["B3v"][:], alpha=1.0)

            # dynamics: -2Ax and g = -2G^T x in SP2, then transpose to xview
            dyP = psM.tile([128, 2, MV], f32, tag="mmP", name="dyP")
            nc.tensor.matmul(dyP[:, 0, :], C["TDA"][:], xsp2[:])
            nc.tensor.matmul(dyP[:, 1, :], C["TDG"][:], xsp2[:])
            dyS = work.tile([128, 2, MV], bf16, tag="dyS", name="dyS")
            SC.activation(dyS[:], dyP[:], AF.Copy)

            trP = psX.tile([128, 2, SUP, 128], bf16, tag="trP", name="trP")
            for t in range(SUP):
                nc.tensor.transpose(trP[:, 0, t, :],
                                    dyS[:, 0, 128 * t:128 * t + 128],
                                    C["ID128H"][:])
                nc.tensor.transpose(trP[:, 1, t, :],
                                    dyS[:, 1, 128 * t:128 * t + 128],
                                    C["ID128H"][:])
            # prodA = (-2Ax)_xv * x_xv ; g_xv evac ; sqx = x*x
            V.tensor_tensor(prodA[:, cs].rearrange("p (t c) -> p t c", t=SUP),
                            trP[:, 0, :, :],
                            xvb[:, cs].rearrange("p (t c) -> p t c", t=SUP),
                            AL.mult)
            SC.activation(g_xv[:, cs].rearrange("p (t c) -> p t c", t=SUP),
                          trP[:, 1, :, :], AF.Copy)
            GP.tensor_tensor(sqx[:, cs], xvb[:, cs], xvb[:, cs], AL.mult)

            # px/alpha transpose back: pxal (128, 2, MV) -> 2*SUP blocks
            paT = psX.tile([128, 2 * SUP, 128], bf16, tag="paT", name="paT")
            for k in range(2):
                for t in range(SUP):
                    nc.tensor.transpose(paT[:, SUP * k + t, :],
                                        pxal[:, k, 128 * t:128 * t + 128],
                                        C["ID128H"][:])
            # px_xv[r, 128t+64k+16g+8s0+j] = paT[r, SUP*k+t, 32g+16s0+j]
            dstp = px_xv[:, cs].rearrange("p (t k g s j) -> p t k g s j",
                                          t=SUP, k=2, g=4, s=2, j=8)
            srcp = paT.rearrange("p (k t) (g s m) -> p t k g s m",
                                 k=2, g=4, s=2, m=16)[:, :, :, :, :, 0:8]
            V.tensor_copy(dstp, srcp)
            dsta = araw[:, ss].rearrange("p (t k g s one) -> p t k g s one",
                                         t=SUP, k=2, g=4, s=2, one=1)
            srca = paT.rearrange("p (k t) (g s m) -> p t k g s m",
                                 k=2, g=4, s=2, m=16)[:, :, :, :, :, 8:9]
            GP.tensor_copy(dsta, srca)

        # ---------------- Phase B setup (per chunk) ----------------
        def setup_chunk(ch):
            fs = slice(CF * ch, CF * ch + CF)
            sl = slice(CL * ch, CL * ch + CL)
            # c0 = lfh + 4*sigmoid(araw)*(16 - sxx)
            tree8(prodA, lfh, sl)
            tree8(sqx, sxx, sl)
            SC.activation(al4[:, sl], araw[:, sl], AF.Sigmoid)
            GP.tensor_scalar(sxx[:, sl], sxx[:, sl], -1.0, 16.0, AL.mult, AL.add)
            GP.scalar_tensor_tensor(c0s[:, sl], al4[:, sl], 4.0, sxx[:, sl],
                                    AL.mult, AL.mult)
            GP.tensor_tensor(c0s[:, sl], c0s[:, sl], lfh[:, sl], AL.add)
            # transform: gt = |g|, pt = sign(g)*p, q = gt^2
            SC.activation(gt[:, fs], g_xv[:, fs], AF.Abs)
            SC.activation(sgn[:, fs], g_xv[:, fs], AF.Sign)
            V.tensor_tensor(pt[:, fs], sgn[:, fs], px_xv[:, fs], AL.mult)
            V.tensor_tensor(qq[:, fs], gt[:, fs], gt[:, fs], AL.mult)
            # init lam = clip(-(c0 - sum gt*pt)/(sum q + eps), 0, LAMCAP)
            V.tensor_tensor(rb[:, fs], gt[:, fs], pt[:, fs], AL.mult)
            tree8(rb, csum, sl)
            tree8(qq, dsum, sl)
            GP.tensor_tensor(cc[:, sl], c0s[:, sl], csum[:, sl], AL.subtract)
            GP.tensor_scalar(dsum[:, sl], dsum[:, sl], EPS, None, AL.add)
            V.reciprocal(rr[:, sl], dsum[:, sl])
            GP.tensor_tensor(st[:, sl], cc[:, sl], rr[:, sl], AL.mult)
            GP.tensor_scalar(lam[:, sl], st[:, sl], -1.0, 0.0, AL.mult, AL.max)
            GP.tensor_scalar(lam[:, sl], lam[:, sl], LAMCAP, None, AL.min)
            V.tensor_copy(lam2[:, sl, :],
                          lam[:, sl, None].broadcast_to((128, CL, 2)))

        def l2v(ap_pair, sl):
            # dup-pair bf16 slot view broadcast to (128, CL, 4, 2)
            return ap_pair[:, sl, None, :].broadcast_to((128, CL, 4, 2))

        def xpair(ap, sl):
            fs = slice(sl.start * 8, sl.stop * 8)
            return ap[:, fs].rearrange("p (c k two) -> p c k two", k=4, two=2)

        def iter_chunk(ch):
            sl = slice(CL * ch, CL * ch + CL)
            fs = slice(CF * ch, CF * ch + CF)
            V.tensor_tensor(xpair(ur, sl), l2v(lam2, sl), xpair(gt, sl), AL.mult)
            V.tensor_tensor(ur[:, fs], ur[:, fs], pt[:, fs], AL.subtract)
            GP.tensor_scalar(uh[:, fs], ur[:, fs], 1.0, -1.0, AL.min, AL.max)
            V.tensor_tensor(rb[:, fs], gt[:, fs], uh[:, fs], AL.mult)
            tree8(rb, csum, sl)
            GP.tensor_tensor(cc[:, sl], c0s[:, sl], csum[:, sl], AL.add)
            SC.activation(bvs2[:, sl, :],
                          cc[:, sl, None].broadcast_to((128, CL, 2)),
                          AF.Sign, scale=-1.0)
            V.tensor_tensor(xpair(nf, sl), xpair(uh, sl), l2v(bvs2, sl),
                            AL.not_equal)
            V.tensor_tensor(qq[:, fs], qq[:, fs], nf[:, fs], AL.mult)
            tree8(qq, dsum, sl)
            GP.tensor_scalar(dsum[:, sl], dsum[:, sl], EPS, None, AL.add)
            V.reciprocal(rr[:, sl], dsum[:, sl])
            GP.tensor_tensor(st[:, sl], cc[:, sl], rr[:, sl], AL.mult)
            GP.tensor_tensor(lam[:, sl], lam[:, sl], st[:, sl], AL.subtract)
            GP.tensor_scalar(lam[:, sl], lam[:, sl], 0.0, LAMCAP, AL.max, AL.min)
            V.tensor_copy(lam2[:, sl, :],
                          lam[:, sl, None].broadcast_to((128, CL, 2)))

        def final_chunk(ch):
            sl = slice(CL * ch, CL * ch + CL)
            fs = slice(CF * ch, CF * ch + CF)
            V.tensor_tensor(xpair(ur, sl), l2v(lam2, sl), xpair(g_xv, sl),
                            AL.mult)
            V.tensor_tensor(ur[:, fs], ur[:, fs], px_xv[:, fs], AL.subtract)
            V.tensor_scalar(uout[:, fs], ur[:, fs], 1.0, -1.0, AL.min, AL.max)
            for tt in range(8 * ch, 8 * (ch + 1)):
                nc.sync.dma_start(
                    u_d[tt * TILE:(tt + 1) * TILE, :].rearrange(
                        "(b r s) j -> r b s j", b=8, r=128, s=2),
                    uout[:, 128 * tt:128 * tt + 128].rearrange(
                        "p (b s j) -> p b s j", b=8, s=2, j=8))

        # ---------------- emission order (pipelined) ----------------
        for ch in range(NCH):
            for s in range(SPC):
                phase_a(SPC * ch + s)
            setup_chunk(ch)
        for it in range(T_NEWTON):
            for ch in range(NCH):
                iter_chunk(ch)
        for ch in range(NCH):
            final_chunk(ch)


def _build():
    from concourse import bacc, mybir
    from concourse import tile as tile_mod
    from concourse._compat import axon_active
    bf16 = mybir.dt.bfloat16
    nc = bacc.Bacc("TRN2", target_bir_lowering=False,
                   debug=not axon_active(), num_devices=NCORES)
    x_d = nc.dram_tensor("x", [S, N], bf16, kind="ExternalInput").ap()
    u_d = nc.dram_tensor("u", [S, N], bf16, kind="ExternalOutput").ap()
    cds = {}
    for k, v in _CSHAPES_BF.items():
        cds[k] = nc.dram_tensor(k, list(v), bf16, kind="ExternalInput").ap()
    for k, v in _CSHAPES_F32.items():
        cds[k] = nc.dram_tensor(k, list(v), mybir.dt.float32,
                                kind="ExternalInput").ap()
    with tile_mod.TileContext(nc) as tc:
        build_kernel(nc, tc, x_d, u_d, cds)
    nc.compile()
    return nc


def kernel(x, W1, b1, W21, b21, W22, b22, W31, b31, W32, b32, A, G, mean, std):
    import ml_dtypes
    from concourse.bass_utils import run_bass_kernel_spmd
    f32 = np.float32
    bf = ml_dtypes.bfloat16
    x = np.asarray(x, f32)
    x0 = (x * np.asarray(std, f32) + np.asarray(mean, f32)).astype(bf)

    consts = _consts(np.asarray(W1, f32), np.asarray(b1, f32), np.asarray(W21, f32),
                     np.asarray(b21, f32), np.asarray(W22, f32), np.asarray(b22, f32),
                     np.asarray(W31, f32), np.asarray(b31, f32), np.asarray(W32, f32),
                     np.asarray(b32, f32), np.asarray(A, f32), np.asarray(G, f32))
    if "nc" not in _CACHE:
        _CACHE["nc"] = _build()
    nc = _CACHE["nc"]

    in_maps = []
    for c in range(NCORES):
        m = {"x": np.ascontiguousarray(x0[c * S:(c + 1) * S])}
        m.update(consts)
        in_maps.append(m)
    res = run_bass_kernel_spmd(nc, in_maps, list(range(NCORES)))
    out = np.concatenate([np.asarray(res.results[c]["u"]).astype(f32)
                          for c in range(NCORES)], axis=0)
    return out


# revision 9
# speedup vs baseline: 1.4699x; 1.0521x over previous
"""Trainium2 Bass kernel for nn_BarrierPolicy (CBF-QP safety filter), v2.

Data-parallel over batch: 8 cores x 32768 samples, all math bf16 on-chip.

Phase A (per 4-tile super-block of 8192 samples): x arrives bf16 in xview
layout (partition r, col 16b+8s0+j), PE-transposes to SP2, runs the MLP +
dynamics matmuls with 512-wide moving dim (few, fat matmuls; bf16 PSUM
outputs), evacuates with relu+bias as wide ops split across ACT/Pool/DVE,
transposes px/g/(-2Ax) back to xview in batched PSUM banks.

Phase B (per 8-tile chunk, pipelined behind phase A): Newton-form Kiwiel
variable-fixing for the box-QP dual:
  lam' = clip(lam - c(lam)/den, 0, LAMCAP),  den = sum of q over the
  not-yet-fixed set; coords are permanently fixed one-sided (uhat == bvs,
  bvs = sign(-c)). Clip via 4x tensor_scalar, per-sample reductions via
  2-level bf16 tree-add + f32 final, per-sample scalars broadcast through a
  duplicated-pair view that keeps DVE in 2x mode. T_NEWTON iterations with
  closed-form final u = clip(lam*g - p).
"""
import numpy as np

B_FULL, N = 262144, 8
NCORES = 8
S = B_FULL // NCORES          # 32768 samples per core
TILE = 2048
NT = S // TILE                # 16 tiles
SUP = 4                       # tiles per super-block
NSUP = NT // SUP
MV = 128 * SUP                # matmul moving width per super (512)
FC = S // 16                  # 2048 xview cols per core
NSLOT = S // 128              # 256 slot cols per core
NCH = 2                       # phase-B chunks
CF = FC // NCH                # 1024
CL = NSLOT // NCH             # 128
SPC = NSUP // NCH             # supers per chunk
T_NEWTON = 3
LAMCAP = float(2.0 ** 40)
EPS = 1e-9

_CACHE = {}

_CSHAPES_BF = dict(TL2=(128, 128), TL3S=(128, 32), TDA=(128, 128),
                   TDG=(128, 128), ID128H=(128, 128),
                   **{f"TL1E{b}": (128, 128) for b in range(8)})
_CSHAPES_F32 = dict(B1v=(128, 1), B2v=(128, 1), B3v=(128, 1))


def _consts(W1, b1, W21, b21, W22, b22, W31, b31, W32, b32, A, G):
    import ml_dtypes
    f32 = np.float32
    bf = ml_dtypes.bfloat16
    out = {}
    for b in range(8):
        T = np.zeros((128, 128), f32)
        for s0 in range(2):
            T[16 * b + 8 * s0:16 * b + 8 * s0 + 8, 64 * s0:64 * s0 + 64] = W1
        out[f"TL1E{b}"] = T.astype(bf)
    TL2 = np.zeros((128, 128), f32)
    for s0 in range(2):
        TL2[64 * s0:64 * s0 + 64, 32 * s0:32 * s0 + 32] = W21
        TL2[64 * s0:64 * s0 + 64, 64 + 32 * s0:64 + 32 * s0 + 32] = W22
    # stacked L3: out col m = 16*s0 + mm, mm in 0..7 -> px_j, mm=8 -> alpha raw
    TL3S = np.zeros((128, 32), f32)
    for s0 in range(2):
        TL3S[32 * s0:32 * s0 + 32, 16 * s0:16 * s0 + 8] = W31
        TL3S[64 + 32 * s0:64 + 32 * s0 + 32, 16 * s0 + 8:16 * s0 + 9] = W32
    TDA = np.kron(np.eye(16, dtype=f32), (-2.0 * A.T).astype(f32))  # -2 A x
    TDG = np.kron(np.eye(16, dtype=f32), (-2.0 * G).astype(f32))    # -2 G^T x
    out.update(TL2=TL2.astype(bf), TL3S=TL3S.astype(bf),
               TDA=TDA.astype(bf), TDG=TDG.astype(bf),
               ID128H=np.eye(128, dtype=f32).astype(bf))
    out["B1v"] = np.concatenate([b1, b1]).reshape(128, 1).astype(f32)
    out["B2v"] = np.concatenate([b21, b21, b22, b22]).reshape(128, 1).astype(f32)
    b3 = np.concatenate([b31.astype(f32), [np.float32(b32[0])], np.zeros(7, f32)])
    out["B3v"] = np.tile(b3, 8).reshape(128, 1).astype(f32)
    return out


def build_kernel(nc, tc, x_d, u_d, cds):
    from concourse import mybir
    f32 = mybir.dt.float32
    bf16 = mybir.dt.bfloat16
    AL = mybir.AluOpType
    AF = mybir.ActivationFunctionType
    V, GP, SC = nc.vector, nc.gpsimd, nc.scalar

    with (
        tc.tile_pool(name="const", bufs=1) as cpool,
        tc.tile_pool(name="pers", bufs=1) as pers,
        tc.tile_pool(name="work", bufs=2) as work,
        tc.tile_pool(name="psT", bufs=2, space="PSUM") as psT,
        tc.tile_pool(name="psM", bufs=2, space="PSUM") as psM,
        tc.tile_pool(name="psX", bufs=1, space="PSUM") as psX,
    ):
        C = {}
        for k, v in _CSHAPES_BF.items():
            C[k] = cpool.tile(list(v), bf16, tag=k, name=k)
        for k, v in _CSHAPES_F32.items():
            C[k] = cpool.tile(list(v), f32, tag=k, name=k)
        for k in list(_CSHAPES_BF) + list(_CSHAPES_F32):
            nc.sync.dma_start(C[k][:], cds[k][:])

        def fc_bf(tag):
            return pers.tile([128, FC], bf16, tag=tag, name=tag)

        def sl_f32(tag):
            return pers.tile([128, NSLOT], f32, tag=tag, name=tag)

        # persistent full-width tensors (xview layout)
        xvb = fc_bf("xvb")
        g_xv, px_xv = fc_bf("g_xv"), fc_bf("px_xv")
        gt, pt, qq = fc_bf("gt"), fc_bf("pt"), fc_bf("qq")
        sgn = fc_bf("sgn")
        ur, uh, rb = fc_bf("ur"), fc_bf("uh"), fc_bf("rb")
        nf = fc_bf("nf")
        prodA, sqx = fc_bf("prodA"), fc_bf("sqx")
        uout = fc_bf("uout")
        l1a = pers.tile([128, FC // 2], bf16, tag="l1a", name="l1a")
        l2a = pers.tile([128, FC // 4], bf16, tag="l2a", name="l2a")
        l1b = pers.tile([128, FC // 2], bf16, tag="l1b", name="l1b")
        l2b = pers.tile([128, FC // 4], bf16, tag="l2b", name="l2b")
        # per-sample slots (f32) and dup-pair broadcasts (bf16)
        c0s, lam, lfh, sxx = sl_f32("c0s"), sl_f32("lam"), sl_f32("lfh"), sl_f32("sxx")
        csum, dsum, cc = sl_f32("csum"), sl_f32("dsum"), sl_f32("cc")
        rr, st = sl_f32("rr"), sl_f32("st")
        araw = pers.tile([128, NSLOT], bf16, tag="araw", name="araw")
        al4 = pers.tile([128, NSLOT], bf16, tag="al4", name="al4")
        lam2 = pers.tile([128, NSLOT, 2], bf16, tag="lam2", name="lam2")
        bvs2 = pers.tile([128, NSLOT, 2], bf16, tag="bvs2", name="bvs2")

        x8 = lambda ap: ap.rearrange("p (c j) -> p c j", j=8)
        x4 = lambda ap: ap.rearrange("p (c j) -> p c j", j=4)
        x2v = lambda ap: ap.rearrange("p (c j) -> p c j", j=2)

        def tree8(src, out_f32, sl, eng_last=GP):
            """out[:, sl] = sum over j=8 of src[:, 8*sl]: 2 bf16 levels + f32."""
            fs = slice(sl.start * 8, sl.stop * 8)
            h1s = slice(sl.start * 4, sl.stop * 4)
            h2s = slice(sl.start * 2, sl.stop * 2)
            la, lb = (l1a, l2a) if src is not qq else (l1b, l2b)
            s8 = x8(src[:, fs])
            V.tensor_tensor(x4(la[:, h1s]), s8[:, :, 0:4], s8[:, :, 4:8], AL.add)
            V.tensor_tensor(x2v(lb[:, h2s]), x4(la[:, h1s])[:, :, 0:2],
                            x4(la[:, h1s])[:, :, 2:4], AL.add)
            eng_last.tensor_tensor(out_f32[:, sl], x2v(lb[:, h2s])[:, :, 0],
                                   x2v(lb[:, h2s])[:, :, 1], AL.add)

        # ---------------- Phase A (per super-block) ----------------
        # contiguous layout: partition P, col 8c+j <-> sample 256P + c, coord j
        x_flat = x_d.rearrange("(P c) j -> P (c j)", P=128)
        u_flat = u_d.rearrange("(P c) j -> P (c j)", P=128)

        def phase_a(sp):
            cs = slice(MV * sp, MV * sp + MV)               # xview cols
            ss = slice(16 * SUP * sp, 16 * SUP * (sp + 1))  # slot cols
            nc.sync.dma_start(xvb[:, cs], x_flat[:, cs])
            TPx = psT.tile([128, SUP, 128], bf16, tag="TPx", name="TPx")
            for t in range(SUP):
                tt = SUP * sp + t
                nc.tensor.transpose(TPx[:, t, :], xvb[:, 128 * tt:128 * tt + 128],
                                    C["ID128H"][:])
            xsp2 = work.tile([128, MV], bf16, tag="xsp2", name="xsp2")
            V.tensor_copy(xsp2[:].rearrange("p (t c) -> p t c", t=SUP), TPx[:])

            h1 = work.tile([128, 8, MV], bf16, tag="h1", name="h1")
            x2 = work.tile([128, 8, MV], bf16, tag="x2", name="x2")
            ev1 = [SC, GP, SC, V]
            ev2 = [SC, GP, SC, V]
            for qr in range(4):
                mmP = psM.tile([128, 2, MV], f32, tag="mmP", name="mmP")
                for bi in range(2):
                    nc.tensor.matmul(mmP[:, bi, :], C[f"TL1E{2 * qr + bi}"][:],
                                     xsp2[:])
                hs = slice(2 * qr, 2 * qr + 2)
                e = ev1[qr]
                if e is SC:
                    SC.activation(h1[:, hs, :], mmP[:], AF.Relu, bias=C["B1v"][:])
                else:
                    e.tensor_scalar(h1[:, hs, :], mmP[:], C["B1v"][:], 0.0,
                                    AL.add, AL.max)
            for qr in range(4):
                mmP = psM.tile([128, 2, MV], f32, tag="mmP", name="mmP")
                nc.tensor.matmul(mmP[:].rearrange("p a m -> p (a m)"), C["TL2"][:],
                                 h1[:, 2 * qr:2 * qr + 2, :].rearrange(
                                     "p a m -> p (a m)"))
                hs = slice(2 * qr, 2 * qr + 2)
                e = ev2[qr]
                if e is SC:
                    SC.activation(x2[:, hs, :], mmP[:], AF.Relu, bias=C["B2v"][:])
                else:
                    e.tensor_scalar(x2[:, hs, :], mmP[:], C["B2v"][:], 0.0,
                                    AL.add, AL.max)

            # L3 pairs: b = 2g+k -> out partitions 32g+16s0+mm, psum slot k
            LA = psM.tile([128, 2, MV], f32, tag="mmP", name="LA")
            for g4 in range(4):
                nc.tensor.matmul(LA[32 * g4:32 * g4 + 32, :, :].rearrange(
                    "p a m -> p (a m)"), C["TL3S"][:],
                    x2[:, 2 * g4:2 * g4 + 2, :].rearrange("p a m -> p (a m)"),
                    tile_position=(0, 32 * g4))
            pxal = work.tile([128, 2, MV], bf16, tag="pxal", name="pxal")
            GP.tensor_scalar(pxal[:], LA[:], C["B3v"][:], None, AL.add)

            # dynamics: -2Ax and g = -2G^T x in SP2, then transpose to xview
            dyP = psM.tile([128, 2, MV], f32, tag="mmP", name="dyP")
            nc.tensor.matmul(dyP[:, 0, :], C["TDA"][:], xsp2[:])
            nc.tensor.matmul(dyP[:, 1, :], C["TDG"][:], xsp2[:])
            dyS = work.tile([128, 2, MV], bf16, tag="dyS", name="dyS")
            SC.activation(dyS[:], dyP[:], AF.Copy)

            trP = psX.tile([128, 2, SUP, 128], bf16, tag="trP", name="trP")
            for t in range(SUP):
                nc.tensor.transpose(trP[:, 0, t, :],
                                    dyS[:, 0, 128 * t:128 * t + 128],
                                    C["ID128H"][:])
                nc.tensor.transpose(trP[:, 1, t, :],
                                    dyS[:, 1, 128 * t:128 * t + 128],
                                    C["ID128H"][:])
            # prodA = (-2Ax)_xv * x_xv ; g_xv evac ; sqx = x*x
            V.tensor_tensor(prodA[:, cs].rearrange("p (t c) -> p t c", t=SUP),
                            trP[:, 0, :, :],
                            xvb[:, cs].rearrange("p (t c) -> p t c", t=SUP),
                            AL.mult)
            SC.activation(g_xv[:, cs].rearrange("p (t c) -> p t c", t=SUP),
                          trP[:, 1, :, :], AF.Copy)
            GP.tensor_tensor(sqx[:, cs], xvb[:, cs], xvb[:, cs], AL.mult)

            # px/alpha transpose back: pxal (128, 2, MV) -> 2*SUP blocks
            paT = psX.tile([128, 2 * SUP, 128], bf16, tag="paT", name="paT")
            for k in range(2):
                for t in range(SUP):
                    nc.tensor.transpose(paT[:, SUP * k + t, :],
                                        pxal[:, k, 128 * t:128 * t + 128],
                                        C["ID128H"][:])
            # px_xv[r, 128t+32g+16k+8s0+j] = paT[r, SUP*k+t, 32g+16s0+j]
            dstp = px_xv[:, cs].rearrange("p (t g k s j) -> p t k g s j",
                                          t=SUP, k=2, g=4, s=2, j=8)
            srcp = paT.rearrange("p (k t) (g s m) -> p t k g s m",
                                 k=2, g=4, s=2, m=16)[:, :, :, :, :, 0:8]
            V.tensor_copy(dstp, srcp)
            dsta = araw[:, ss].rearrange("p (t g k s one) -> p t k g s one",
                                         t=SUP, g=4, k=2, s=2, one=1)
            srca = paT.rearrange("p (k t) (g s m) -> p t k g s m",
                                 k=2, g=4, s=2, m=16)[:, :, :, :, :, 8:9]
            GP.tensor_copy(dsta, srca)

        # ---------------- Phase B setup (per chunk) ----------------
        def setup_chunk(ch):
            fs = slice(CF * ch, CF * ch + CF)
            sl = slice(CL * ch, CL * ch + CL)
            # c0 = lfh + 4*sigmoid(araw)*(16 - sxx)
            tree8(prodA, lfh, sl)
            tree8(sqx, sxx, sl)
            SC.activation(al4[:, sl], araw[:, sl], AF.Sigmoid)
            GP.tensor_scalar(sxx[:, sl], sxx[:, sl], -1.0, 16.0, AL.mult, AL.add)
            GP.scalar_tensor_tensor(c0s[:, sl], al4[:, sl], 4.0, sxx[:, sl],
                                    AL.mult, AL.mult)
            GP.tensor_tensor(c0s[:, sl], c0s[:, sl], lfh[:, sl], AL.add)
            # transform: gt = |g|, pt = sign(g)*p, q = gt^2
            SC.activation(gt[:, fs], g_xv[:, fs], AF.Abs)
            SC.activation(sgn[:, fs], g_xv[:, fs], AF.Sign)
            V.tensor_tensor(pt[:, fs], sgn[:, fs], px_xv[:, fs], AL.mult)
            V.tensor_tensor(qq[:, fs], gt[:, fs], gt[:, fs], AL.mult)
            # init lam = clip(-(c0 - sum gt*pt)/(sum q + eps), 0, LAMCAP)
            V.tensor_tensor(rb[:, fs], gt[:, fs], pt[:, fs], AL.mult)
            tree8(rb, csum, sl)
            tree8(qq, dsum, sl)
            GP.tensor_tensor(cc[:, sl], c0s[:, sl], csum[:, sl], AL.subtract)
            GP.tensor_scalar(dsum[:, sl], dsum[:, sl], EPS, None, AL.add)
            V.reciprocal(rr[:, sl], dsum[:, sl])
            GP.tensor_tensor(st[:, sl], cc[:, sl], rr[:, sl], AL.mult)
            GP.tensor_scalar(lam[:, sl], st[:, sl], -1.0, 0.0, AL.mult, AL.max)
            GP.tensor_scalar(lam[:, sl], lam[:, sl], LAMCAP, None, AL.min)
            V.tensor_copy(lam2[:, sl, :],
                          lam[:, sl, None].broadcast_to((128, CL, 2)))

        def l2v(ap_pair, sl):
            # dup-pair bf16 slot view broadcast to (128, CL, 4, 2)
            return ap_pair[:, sl, None, :].broadcast_to((128, CL, 4, 2))

        def xpair(ap, sl):
            fs = slice(sl.start * 8, sl.stop * 8)
            return ap[:, fs].rearrange("p (c k two) -> p c k two", k=4, two=2)

        def iter_chunk(ch):
            sl = slice(CL * ch, CL * ch + CL)
            fs = slice(CF * ch, CF * ch + CF)
            V.tensor_tensor(xpair(ur, sl), l2v(lam2, sl), xpair(gt, sl), AL.mult)
            V.tensor_tensor(ur[:, fs], ur[:, fs], pt[:, fs], AL.subtract)
            GP.tensor_scalar(uh[:, fs], ur[:, fs], 1.0, -1.0, AL.min, AL.max)
            V.tensor_tensor(rb[:, fs], gt[:, fs], uh[:, fs], AL.mult)
            tree8(rb, csum, sl)
            GP.tensor_tensor(cc[:, sl], c0s[:, sl], csum[:, sl], AL.add)
            SC.activation(bvs2[:, sl, :],
                          cc[:, sl, None].broadcast_to((128, CL, 2)),
                          AF.Sign, scale=-1.0)
            V.tensor_tensor(xpair(nf, sl), xpair(uh, sl), l2v(bvs2, sl),
                            AL.not_equal)
            V.tensor_tensor(qq[:, fs], qq[:, fs], nf[:, fs], AL.mult)
            tree8(qq, dsum, sl)
            GP.tensor_scalar(dsum[:, sl], dsum[:, sl], EPS, None, AL.add)
            V.reciprocal(rr[:, sl], dsum[:, sl])
            GP.tensor_tensor(st[:, sl], cc[:, sl], rr[:, sl], AL.mult)
            GP.tensor_tensor(lam[:, sl], lam[:, sl], st[:, sl], AL.subtract)
            GP.tensor_scalar(lam[:, sl], lam[:, sl], 0.0, LAMCAP, AL.max, AL.min)
            V.tensor_copy(lam2[:, sl, :],
                          lam[:, sl, None].broadcast_to((128, CL, 2)))

        def final_chunk(ch):
            sl = slice(CL * ch, CL * ch + CL)
            fs = slice(CF * ch, CF * ch + CF)
            V.tensor_tensor(xpair(ur, sl), l2v(lam2, sl), xpair(g_xv, sl),
                            AL.mult)
            V.tensor_tensor(ur[:, fs], ur[:, fs], px_xv[:, fs], AL.subtract)
            V.tensor_scalar(uout[:, fs], ur[:, fs], 1.0, -1.0, AL.min, AL.max)
            nc.sync.dma_start(u_flat[:, fs], uout[:, fs])

        # ---------------- emission order (pipelined) ----------------
        for ch in range(NCH):
            for s in range(SPC):
                phase_a(SPC * ch + s)
            setup_chunk(ch)
        for it in range(T_NEWTON):
            for ch in range(NCH):
                iter_chunk(ch)
        for ch in range(NCH):
            final_chunk(ch)


def _build():
    from concourse import bacc, mybir
    from concourse import tile as tile_mod
    from concourse._compat import axon_active
    bf16 = mybir.dt.bfloat16
    nc = bacc.Bacc("TRN2", target_bir_lowering=False,
                   debug=not axon_active(), num_devices=NCORES)
    x_d = nc.dram_tensor("x", [S, N], bf16, kind="ExternalInput").ap()
    u_d = nc.dram_tensor("u", [S, N], bf16, kind="ExternalOutput").ap()
    cds = {}
    for k, v in _CSHAPES_BF.items():
        cds[k] = nc.dram_tensor(k, list(v), bf16, kind="ExternalInput").ap()
    for k, v in _CSHAPES_F32.items():
        cds[k] = nc.dram_tensor(k, list(v), mybir.dt.float32,
                                kind="ExternalInput").ap()
    with tile_mod.TileContext(nc) as tc:
        build_kernel(nc, tc, x_d, u_d, cds)
    nc.compile()
    return nc


def kernel(x, W1, b1, W21, b21, W22, b22, W31, b31, W32, b32, A, G, mean, std):
    import ml_dtypes
    from concourse.bass_utils import run_bass_kernel_spmd
    f32 = np.float32
    bf = ml_dtypes.bfloat16
    x = np.asarray(x, f32)
    x0 = (x * np.asarray(std, f32) + np.asarray(mean, f32)).astype(bf)

    consts = _consts(np.asarray(W1, f32), np.asarray(b1, f32), np.asarray(W21, f32),
                     np.asarray(b21, f32), np.asarray(W22, f32), np.asarray(b22, f32),
                     np.asarray(W31, f32), np.asarray(b31, f32), np.asarray(W32, f32),
                     np.asarray(b32, f32), np.asarray(A, f32), np.asarray(G, f32))
    if "nc" not in _CACHE:
        _CACHE["nc"] = _build()
    nc = _CACHE["nc"]

    in_maps = []
    for c in range(NCORES):
        m = {"x": np.ascontiguousarray(x0[c * S:(c + 1) * S])}
        m.update(consts)
        in_maps.append(m)
    res = run_bass_kernel_spmd(nc, in_maps, list(range(NCORES)))
    out = np.concatenate([np.asarray(res.results[c]["u"]).astype(f32)
                          for c in range(NCORES)], axis=0)
    return out


# revision 20
# speedup vs baseline: 1.5931x; 1.0838x over previous
"""Trainium2 Bass kernel for nn_BarrierPolicy (CBF-QP safety filter), v2.

Data-parallel over batch: 8 cores x 32768 samples, all math bf16 on-chip.

Phase A (per 4-tile super-block of 8192 samples): x arrives bf16 in xview
layout (partition r, col 16b+8s0+j), PE-transposes to SP2, runs the MLP +
dynamics matmuls with 512-wide moving dim (few, fat matmuls; bf16 PSUM
outputs), evacuates with relu+bias as wide ops split across ACT/Pool/DVE,
transposes px/g/(-2Ax) back to xview in batched PSUM banks.

Phase B (per 8-tile chunk, pipelined behind phase A): Newton-form Kiwiel
variable-fixing for the box-QP dual:
  lam' = clip(lam - c(lam)/den, 0, LAMCAP),  den = sum of q over the
  not-yet-fixed set; coords are permanently fixed one-sided (uhat == bvs,
  bvs = sign(-c)). Clip via 4x tensor_scalar, per-sample reductions via
  2-level bf16 tree-add + f32 final, per-sample scalars broadcast through a
  duplicated-pair view that keeps DVE in 2x mode. T_NEWTON iterations with
  closed-form final u = clip(lam*g - p).
"""
import numpy as np

B_FULL, N = 262144, 8
NCORES = 8
S = B_FULL // NCORES          # 32768 samples per core
TILE = 2048
NT = S // TILE                # 16 tiles
SUP = 4                       # tiles per super-block
NSUP = NT // SUP
MV = 128 * SUP                # matmul moving width per super (512)
FC = S // 16                  # 2048 xview cols per core
NSLOT = S // 128              # 256 slot cols per core
NCH = 2                       # phase-B chunks
CF = FC // NCH                # 1024
CL = NSLOT // NCH             # 128
SPC = NSUP // NCH             # supers per chunk
T_NEWTON = 3
LAMCAP = float(2.0 ** 40)
EPS = 1e-9

_CACHE = {}

_CSHAPES_BF = dict(TL2=(128, 128), TL3S=(128, 32), TDA=(128, 128),
                   TDG=(128, 128), ID128H=(128, 128), B31F=(128, 8),
                   **{f"TL1E{b}": (128, 128) for b in range(8)})
_CSHAPES_F32 = dict(B1v=(128, 1), B2v=(128, 1), B32s=(128, 1))


def _consts(W1, b1, W21, b21, W22, b22, W31, b31, W32, b32, A, G):
    import ml_dtypes
    f32 = np.float32
    bf = ml_dtypes.bfloat16
    out = {}
    for b in range(8):
        T = np.zeros((128, 128), f32)
        for s0 in range(2):
            T[16 * b + 8 * s0:16 * b + 8 * s0 + 8, 64 * s0:64 * s0 + 64] = W1
        out[f"TL1E{b}"] = T.astype(bf)
    TL2 = np.zeros((128, 128), f32)
    for s0 in range(2):
        TL2[64 * s0:64 * s0 + 64, 32 * s0:32 * s0 + 32] = W21
        TL2[64 * s0:64 * s0 + 64, 64 + 32 * s0:64 + 32 * s0 + 32] = W22
    # stacked L3: out col m = 16*s0 + mm, mm in 0..7 -> px_j, mm=8 -> alpha raw
    TL3S = np.zeros((128, 32), f32)
    for s0 in range(2):
        TL3S[32 * s0:32 * s0 + 32, 16 * s0:16 * s0 + 8] = W31
        TL3S[64 + 32 * s0:64 + 32 * s0 + 32, 16 * s0 + 8:16 * s0 + 9] = W32
    TDA = np.kron(np.eye(16, dtype=f32), (-2.0 * A.T).astype(f32))  # -2 A x
    TDG = np.kron(np.eye(16, dtype=f32), (-2.0 * G).astype(f32))    # -2 G^T x
    out.update(TL2=TL2.astype(bf), TL3S=TL3S.astype(bf),
               TDA=TDA.astype(bf), TDG=TDG.astype(bf),
               ID128H=np.eye(128, dtype=f32).astype(bf))
    out["B1v"] = np.concatenate([b1, b1]).reshape(128, 1).astype(f32)
    out["B2v"] = np.concatenate([b21, b21, b22, b22]).reshape(128, 1).astype(f32)
    out["B31F"] = np.tile(b31.astype(f32), (128, 1)).astype(bf)
    out["B32s"] = np.full((128, 1), float(b32[0]), f32)
    return out


def build_kernel(nc, tc, x_d, u_d, cds):
    from concourse import mybir
    f32 = mybir.dt.float32
    bf16 = mybir.dt.bfloat16
    AL = mybir.AluOpType
    AF = mybir.ActivationFunctionType
    V, GP, SC = nc.vector, nc.gpsimd, nc.scalar

    with (
        tc.tile_pool(name="const", bufs=1) as cpool,
        tc.tile_pool(name="pers", bufs=1) as pers,
        tc.tile_pool(name="work", bufs=2) as work,
        tc.tile_pool(name="psT", bufs=2, space="PSUM") as psT,
        tc.tile_pool(name="psM", bufs=2, space="PSUM") as psM,
        tc.tile_pool(name="psX", bufs=1, space="PSUM") as psX,
    ):
        C = {}
        for k, v in _CSHAPES_BF.items():
            C[k] = cpool.tile(list(v), bf16, tag=k, name=k)
        for k, v in _CSHAPES_F32.items():
            C[k] = cpool.tile(list(v), f32, tag=k, name=k)
        for k in list(_CSHAPES_BF) + list(_CSHAPES_F32):
            nc.sync.dma_start(C[k][:], cds[k][:])

        def fc_bf(tag):
            return pers.tile([128, FC], bf16, tag=tag, name=tag)

        def sl_f32(tag):
            return pers.tile([128, NSLOT], f32, tag=tag, name=tag)

        # persistent full-width tensors (xview layout)
        xvb = fc_bf("xvb")
        g_xv, px_xv = fc_bf("g_xv"), fc_bf("px_xv")
        gt, pt, qq = fc_bf("gt"), fc_bf("pt"), fc_bf("qq")
        sgn = fc_bf("sgn")
        ur, uh, rb = fc_bf("ur"), fc_bf("uh"), fc_bf("rb")
        nf = fc_bf("nf")
        prodA, sqx = fc_bf("prodA"), fc_bf("sqx")
        uout = fc_bf("uout")
        l1a = pers.tile([128, FC // 2], bf16, tag="l1a", name="l1a")
        l2a = pers.tile([128, FC // 4], bf16, tag="l2a", name="l2a")
        l1b = pers.tile([128, FC // 2], bf16, tag="l1b", name="l1b")
        l2b = pers.tile([128, FC // 4], bf16, tag="l2b", name="l2b")
        # per-sample slots (f32) and dup-pair broadcasts (bf16)
        c0s, lfh, sxx = sl_f32("c0s"), sl_f32("lfh"), sl_f32("sxx")
        csum, dsum, cc = sl_f32("csum"), sl_f32("dsum"), sl_f32("cc")
        st = sl_f32("st")
        araw = pers.tile([128, NSLOT], bf16, tag="araw", name="araw")
        al4 = pers.tile([128, NSLOT], bf16, tag="al4", name="al4")
        lam2 = pers.tile([128, NSLOT, 2], bf16, tag="lam2", name="lam2")
        bvs2 = pers.tile([128, NSLOT, 2], bf16, tag="bvs2", name="bvs2")

        x8 = lambda ap: ap.rearrange("p (c j) -> p c j", j=8)
        x4 = lambda ap: ap.rearrange("p (c j) -> p c j", j=4)
        x2v = lambda ap: ap.rearrange("p (c j) -> p c j", j=2)

        def tree8(src, out_f32, sl, eng_last=GP):
            """out[:, sl] = sum over j=8 of src[:, 8*sl]: 2 bf16 levels + f32."""
            fs = slice(sl.start * 8, sl.stop * 8)
            h1s = slice(sl.start * 4, sl.stop * 4)
            h2s = slice(sl.start * 2, sl.stop * 2)
            la, lb = (l1a, l2a) if src is not qq else (l1b, l2b)
            s8 = x8(src[:, fs])
            V.tensor_tensor(x4(la[:, h1s]), s8[:, :, 0:4], s8[:, :, 4:8], AL.add)
            V.tensor_tensor(x2v(lb[:, h2s]), x4(la[:, h1s])[:, :, 0:2],
                            x4(la[:, h1s])[:, :, 2:4], AL.add)
            eng_last.tensor_tensor(out_f32[:, sl], x2v(lb[:, h2s])[:, :, 0],
                                   x2v(lb[:, h2s])[:, :, 1], AL.add)

        # ---------------- Phase A (per super-block) ----------------
        # contiguous layout: partition P, col 8c+j <-> sample 256P + c, coord j
        x_flat = x_d.rearrange("(P c) j -> P (c j)", P=128)
        u_flat = u_d.rearrange("(P c) j -> P (c j)", P=128)

        def phase_a(sp):
            cs = slice(MV * sp, MV * sp + MV)               # xview cols
            ss = slice(16 * SUP * sp, 16 * SUP * (sp + 1))  # slot cols
            nc.sync.dma_start(xvb[:, cs], x_flat[:, cs])
            TPx = psT.tile([128, SUP, 128], bf16, tag="TPx", name="TPx")
            for t in range(SUP):
                tt = SUP * sp + t
                nc.tensor.transpose(TPx[:, t, :], xvb[:, 128 * tt:128 * tt + 128],
                                    C["ID128H"][:])
            xsp2 = work.tile([128, MV], bf16, tag="xsp2", name="xsp2")
            V.tensor_copy(xsp2[:].rearrange("p (t c) -> p t c", t=SUP), TPx[:])

            h1 = work.tile([128, 8, MV], bf16, tag="h1", name="h1")
            x2 = work.tile([128, 8, MV], bf16, tag="x2", name="x2")
            ev1 = [SC, GP, SC, GP]
            ev2 = [SC, GP, SC, GP]
            for qr in range(4):
                mmP = psM.tile([128, 2, MV], f32, tag="mmP", name="mmP")
                for bi in range(2):
                    nc.tensor.matmul(mmP[:, bi, :], C[f"TL1E{2 * qr + bi}"][:],
                                     xsp2[:])
                hs = slice(2 * qr, 2 * qr + 2)
                e = ev1[qr]
                if e is SC:
                    SC.activation(h1[:, hs, :], mmP[:], AF.Relu, bias=C["B1v"][:])
                else:
                    e.tensor_scalar(h1[:, hs, :], mmP[:], C["B1v"][:], 0.0,
                                    AL.add, AL.max)
            for qr in range(4):
                mmP = psM.tile([128, 2, MV], f32, tag="mmP", name="mmP")
                nc.tensor.matmul(mmP[:].rearrange("p a m -> p (a m)"), C["TL2"][:],
                                 h1[:, 2 * qr:2 * qr + 2, :].rearrange(
                                     "p a m -> p (a m)"))
                hs = slice(2 * qr, 2 * qr + 2)
                e = ev2[qr]
                if e is SC:
                    SC.activation(x2[:, hs, :], mmP[:], AF.Relu, bias=C["B2v"][:])
                else:
                    e.tensor_scalar(x2[:, hs, :], mmP[:], C["B2v"][:], 0.0,
                                    AL.add, AL.max)

            # L3 pairs: b = 2g+k -> out partitions 32g+16s0+mm, psum slot k
            LA = psM.tile([128, 2, MV], f32, tag="mmP", name="LA")
            for g4 in range(4):
                nc.tensor.matmul(LA[32 * g4:32 * g4 + 32, :, :].rearrange(
                    "p a m -> p (a m)"), C["TL3S"][:],
                    x2[:, 2 * g4:2 * g4 + 2, :].rearrange("p a m -> p (a m)"),
                    tile_position=(0, 32 * g4))
            pxal = work.tile([128, 2, MV], bf16, tag="pxal", name="pxal")
            SC.activation(pxal[:], LA[:], AF.Copy)

            # dynamics: -2Ax and g = -2G^T x in SP2, then transpose to xview
            dyP = psM.tile([128, 2, MV], f32, tag="mmP", name="dyP")
            nc.tensor.matmul(dyP[:, 0, :], C["TDA"][:], xsp2[:])
            nc.tensor.matmul(dyP[:, 1, :], C["TDG"][:], xsp2[:])
            dyS = work.tile([128, 2, MV], bf16, tag="dyS", name="dyS")
            SC.activation(dyS[:], dyP[:], AF.Copy)

            trP = psX.tile([128, 2, SUP, 128], bf16, tag="trP", name="trP")
            for t in range(SUP):
                nc.tensor.transpose(trP[:, 0, t, :],
                                    dyS[:, 0, 128 * t:128 * t + 128],
                                    C["ID128H"][:])
                nc.tensor.transpose(trP[:, 1, t, :],
                                    dyS[:, 1, 128 * t:128 * t + 128],
                                    C["ID128H"][:])
            # prodA = (-2Ax)_xv * x_xv ; g_xv evac ; sqx = x*x
            V.tensor_tensor(prodA[:, cs].rearrange("p (t c) -> p t c", t=SUP),
                            trP[:, 0, :, :],
                            xvb[:, cs].rearrange("p (t c) -> p t c", t=SUP),
                            AL.mult)
            SC.activation(g_xv[:, cs].rearrange("p (t c) -> p t c", t=SUP),
                          trP[:, 1, :, :], AF.Copy)
            SC.activation(sqx[:, cs], xvb[:, cs], AF.Square)

            # px/alpha transpose back: pxal (128, 2, MV) -> 2*SUP blocks
            paT = psX.tile([128, 2 * SUP, 128], bf16, tag="paT", name="paT")
            for k in range(2):
                for t in range(SUP):
                    nc.tensor.transpose(paT[:, SUP * k + t, :],
                                        pxal[:, k, 128 * t:128 * t + 128],
                                        C["ID128H"][:])
            # px_xv[r, 128t+32g+16k+8s0+j] = paT[r, SUP*k+t, 32g+16s0+j] + b31[j]
            dstp = px_xv[:, cs].rearrange("p (t g k s j) -> p t k g s j",
                                          t=SUP, k=2, g=4, s=2, j=8)
            srcp = paT.rearrange("p (k t) (g s m) -> p t k g s m",
                                 k=2, g=4, s=2, m=16)[:, :, :, :, :, 0:8]
            V.tensor_tensor(dstp, srcp,
                            C["B31F"][:, None, None, None, None, :].broadcast_to(
                                (128, SUP, 2, 4, 2, 8)), AL.add)
            dsta = araw[:, ss].rearrange("p (t g k s one) -> p t k g s one",
                                         t=SUP, g=4, k=2, s=2, one=1)
            srca = paT.rearrange("p (k t) (g s m) -> p t k g s m",
                                 k=2, g=4, s=2, m=16)[:, :, :, :, :, 8:9]
            GP.tensor_copy(dsta, srca)

        # ---------------- Phase B setup (per chunk) ----------------
        def setup_chunk(ch):
            fs = slice(CF * ch, CF * ch + CF)
            sl = slice(CL * ch, CL * ch + CL)
            # c0 = lfh + 4*sigmoid(araw + b32)*(16 - sxx)
            tree8(prodA, lfh, sl)
            tree8(sqx, sxx, sl)
            SC.activation(al4[:, sl], araw[:, sl], AF.Sigmoid, bias=C["B32s"][:])
            GP.tensor_scalar(sxx[:, sl], sxx[:, sl], -1.0, 16.0, AL.mult, AL.add)
            GP.scalar_tensor_tensor(c0s[:, sl], al4[:, sl], 4.0, sxx[:, sl],
                                    AL.mult, AL.mult)
            GP.tensor_tensor(c0s[:, sl], c0s[:, sl], lfh[:, sl], AL.add)
            # transform: gt = |g|, pt = sign(g)*p, q = gt^2
            SC.activation(gt[:, fs], g_xv[:, fs], AF.Abs)
            SC.activation(sgn[:, fs], g_xv[:, fs], AF.Sign)
            V.tensor_tensor(pt[:, fs], sgn[:, fs], px_xv[:, fs], AL.mult)
            V.tensor_tensor(qq[:, fs], gt[:, fs], gt[:, fs], AL.mult)
            # init lam = clip(-(c0 - sum gt*pt)/(sum q + eps), 0, LAMCAP)
            V.tensor_tensor(rb[:, fs], gt[:, fs], pt[:, fs], AL.mult)
            tree8(rb, csum, sl, V)
            tree8(qq, dsum, sl, V)
            V.tensor_tensor(cc[:, sl], c0s[:, sl], csum[:, sl], AL.subtract)
            V.tensor_scalar(dsum[:, sl], dsum[:, sl], EPS, None, AL.add)
            V.tensor_tensor(st[:, sl], cc[:, sl], dsum[:, sl], AL.divide)
            V.tensor_scalar(st[:, sl], st[:, sl], -1.0, 0.0, AL.mult, AL.max)
            V.tensor_scalar(lam2[:, sl, :],
                            st[:, sl, None].broadcast_to((128, CL, 2)),
                            LAMCAP, None, AL.min)

        def l2v(ap_pair, sl):
            # dup-pair bf16 slot view broadcast to (128, CL, 4, 2)
            return ap_pair[:, sl, None, :].broadcast_to((128, CL, 4, 2))

        def xpair(ap, sl):
            fs = slice(sl.start * 8, sl.stop * 8)
            return ap[:, fs].rearrange("p (c k two) -> p c k two", k=4, two=2)

        def iter_chunk(ch):
            sl = slice(CL * ch, CL * ch + CL)
            fs = slice(CF * ch, CF * ch + CF)
            V.tensor_tensor(xpair(ur, sl), l2v(lam2, sl), xpair(gt, sl), AL.mult)
            V.tensor_tensor(ur[:, fs], ur[:, fs], pt[:, fs], AL.subtract)
            V.tensor_scalar(uh[:, fs], ur[:, fs], 1.0, -1.0, AL.min, AL.max)
            V.tensor_tensor(rb[:, fs], gt[:, fs], uh[:, fs], AL.mult)
            tree8(rb, csum, sl, V)
            V.tensor_tensor(cc[:, sl], c0s[:, sl], csum[:, sl], AL.add)
            # bvs = -sign-ish(c): +1 if c < 0 else -1 (c==0 -> -1, fixes low)
            V.tensor_scalar(st[:, sl], cc[:, sl], 0.0, None, AL.is_ge)
            V.tensor_scalar(bvs2[:, sl, :],
                            st[:, sl, None].broadcast_to((128, CL, 2)),
                            -2.0, 1.0, AL.mult, AL.add)
            V.tensor_tensor(xpair(nf, sl), xpair(uh, sl), l2v(bvs2, sl),
                            AL.not_equal)
            V.tensor_tensor(qq[:, fs], qq[:, fs], nf[:, fs], AL.mult)
            tree8(qq, dsum, sl, V)
            V.tensor_scalar(dsum[:, sl], dsum[:, sl], EPS, None, AL.add)
            V.tensor_tensor(st[:, sl], cc[:, sl], dsum[:, sl], AL.divide)
            V.tensor_tensor(lam2[:, sl, :], lam2[:, sl, :],
                            st[:, sl, None].broadcast_to((128, CL, 2)),
                            AL.subtract)
            V.tensor_scalar(lam2[:, sl, :], lam2[:, sl, :], 0.0, LAMCAP,
                            AL.max, AL.min)

        def final_chunk(ch):
            sl = slice(CL * ch, CL * ch + CL)
            fs = slice(CF * ch, CF * ch + CF)
            V.tensor_tensor(xpair(ur, sl), l2v(lam2, sl), xpair(g_xv, sl),
                            AL.mult)
            V.tensor_tensor(ur[:, fs], ur[:, fs], px_xv[:, fs], AL.subtract)
            V.tensor_scalar(uout[:, fs], ur[:, fs], 1.0, -1.0, AL.min, AL.max)
            nc.sync.dma_start(u_flat[:, fs], uout[:, fs])

        # ---------------- emission order (pipelined) ----------------
        for ch in range(NCH):
            for s in range(SPC):
                phase_a(SPC * ch + s)
            setup_chunk(ch)
        for it in range(T_NEWTON):
            for ch in range(NCH):
                iter_chunk(ch)
        for ch in range(NCH):
            final_chunk(ch)


def _build():
    from concourse import bacc, mybir
    from concourse import tile as tile_mod
    from concourse._compat import axon_active
    bf16 = mybir.dt.bfloat16
    nc = bacc.Bacc("TRN2", target_bir_lowering=False,
                   debug=not axon_active(), num_devices=NCORES)
    x_d = nc.dram_tensor("x", [S, N], bf16, kind="ExternalInput").ap()
    u_d = nc.dram_tensor("u", [S, N], bf16, kind="ExternalOutput").ap()
    cds = {}
    for k, v in _CSHAPES_BF.items():
        cds[k] = nc.dram_tensor(k, list(v), bf16, kind="ExternalInput").ap()
    for k, v in _CSHAPES_F32.items():
        cds[k] = nc.dram_tensor(k, list(v), mybir.dt.float32,
                                kind="ExternalInput").ap()
    with tile_mod.TileContext(nc) as tc:
        build_kernel(nc, tc, x_d, u_d, cds)
    nc.compile()
    return nc


def kernel(x, W1, b1, W21, b21, W22, b22, W31, b31, W32, b32, A, G, mean, std):
    import ml_dtypes
    from concourse.bass_utils import run_bass_kernel_spmd
    f32 = np.float32
    bf = ml_dtypes.bfloat16
    x = np.asarray(x, f32)
    x0 = (x * np.asarray(std, f32) + np.asarray(mean, f32)).astype(bf)

    consts = _consts(np.asarray(W1, f32), np.asarray(b1, f32), np.asarray(W21, f32),
                     np.asarray(b21, f32), np.asarray(W22, f32), np.asarray(b22, f32),
                     np.asarray(W31, f32), np.asarray(b31, f32), np.asarray(W32, f32),
                     np.asarray(b32, f32), np.asarray(A, f32), np.asarray(G, f32))
    if "nc" not in _CACHE:
        _CACHE["nc"] = _build()
    nc = _CACHE["nc"]

    in_maps = []
    for c in range(NCORES):
        m = {"x": np.ascontiguousarray(x0[c * S:(c + 1) * S])}
        m.update(consts)
        in_maps.append(m)
    res = run_bass_kernel_spmd(nc, in_maps, list(range(NCORES)))
    out = np.concatenate([np.asarray(res.results[c]["u"]).astype(f32)
                          for c in range(NCORES)], axis=0)
    return out
